# revision 5
# baseline (speedup 1.0000x reference)
"""MoE layer (top-2 routing, 8 experts) on 8 TRN2 NeuronCores.

Strategy: expert-parallel. The host computes routing (router matmul, softmax,
top-2, capacity dispatch — cheap integer/index work) and packs each expert's
tokens densely. Core e runs expert e's MLP (fc -> relu^2 -> proj) over its
padded token batch with fp32r matmuls. The host then gathers, weights and
combines the expert outputs.

All tensors are laid out transposed ([feature, token]) so the device kernel
needs no on-chip transposes: both matmuls contract over the partition dim.
"""
import contextlib
import ctypes
import os

import numpy as np

B, T, C, E, H, K = 4, 2048, 1024, 8, 2048, 2
N = B * T
CAP = 2 * N * K // E  # per-expert capacity; overflow tokens are dropped
NBLK = 512            # token block (matmul moving free dim)
P = 128

_NC_CACHE = {}


def _block_widths(max_kept):
    """Token-block widths covering max_kept: full 512s plus a 256 tail if it fits."""
    full, rem = divmod(max(int(max_kept), 1), NBLK)
    if rem == 0:
        return [NBLK] * full
    if rem <= NBLK // 2:
        return [NBLK] * full + [NBLK // 2]
    return [NBLK] * (full + 1)


def _build_nc(blocks, mode):
    from concourse import bacc, mybir, tile

    f32 = mybir.dt.float32
    cd = {"f32r": mybir.dt.float32r, "bf16": mybir.dt.bfloat16,
          "f32": mybir.dt.float32}[mode]
    io_dt = f32 if mode in ("f32r", "f32") else mybir.dt.bfloat16
    KC = C // P   # k-tiles for fc (contract over C)
    KH = H // P   # k-tiles for proj (contract over H)
    RT = sum(blocks)

    def rcast(ap):
        return ap.bitcast(cd) if mode == "f32r" else ap

    nc = bacc.Bacc("TRN2", target_bir_lowering=False, debug=False)
    xeT = nc.dram_tensor("xeT", [C, RT], io_dt, kind="ExternalInput")
    fcwT = nc.dram_tensor("fcwT", [C, H], io_dt, kind="ExternalInput")
    pjwT = nc.dram_tensor("pjwT", [H, C], io_dt, kind="ExternalInput")
    outT = nc.dram_tensor("outT", [C, RT], f32, kind="ExternalOutput")

    xeT_r = rcast(xeT[:].rearrange("(k p) n -> p k n", p=P))
    fcw_r = rcast(fcwT[:].rearrange("(k p) h -> p k h", p=P))
    pjw_r = rcast(pjwT[:].rearrange("(k p) c -> p k c", p=P))
    outT_r = outT[:].rearrange("(m p) n -> p m n", p=P)

    with tile.TileContext(nc) as tc:
        with (
            tc.tile_pool(name="wpool", bufs=1) as wpool,
            tc.tile_pool(name="xpool", bufs=2) as xpool,
            tc.tile_pool(name="hpool", bufs=1) as hpool,
            tc.tile_pool(name="tpool", bufs=3) as tpool,
            tc.tile_pool(name="opool", bufs=3) as opool,
            tc.tile_pool(name="pspool", bufs=4, space="PSUM") as pspool,
        ):
            fcw_sb = wpool.tile([P, KC, H], cd)
            pjw_sb = wpool.tile([P, KH, C], cd)
            x0_sb = xpool.tile([P, KC, NBLK], cd, tag="x")
            w0 = blocks[0]
            dma_q = [nc.sync, nc.scalar]
            # Issue order sets priority within each HWDGE queue: block-0
            # activations first (small), then fc weights, then proj weights.
            # Chunks alternate between the two queues to use both DMA paths.
            for k in range(KC):
                dma_q[k % 2].dma_start(out=x0_sb[:, k, :w0],
                                       in_=xeT_r[:, k, 0:w0])
            for k in range(KC):
                dma_q[k % 2].dma_start(out=fcw_sb[:, k, :], in_=fcw_r[:, k, :])
            for k in range(KH):
                dma_q[k % 2].dma_start(out=pjw_sb[:, k, :], in_=pjw_r[:, k, :])

            def mm1(ps, x_sb, width, m, k):
                nc.tensor.matmul(
                    ps[:, :width],
                    lhsT=fcw_sb[:, k, m * P:(m + 1) * P],
                    rhs=x_sb[:, k, :width],
                    start=(k == 0), stop=(k == KC - 1),
                )

            def mm2(ps, hid, width, m2, k2):
                nc.tensor.matmul(
                    ps[:, :width],
                    lhsT=pjw_sb[:, k2, m2 * P:(m2 + 1) * P],
                    rhs=hid[:, k2, :width],
                    start=(k2 == 0), stop=(k2 == KH - 1),
                )

            def relu_sq(ps, hid, width, m):
                tmp = tpool.tile([P, NBLK], f32, tag="tmp")
                nc.scalar.activation(
                    tmp[:, :width], ps[:, :width],
                    mybir.ActivationFunctionType.Relu,
                )
                nc.vector.tensor_tensor(
                    out=hid[:, m, :width], in0=tmp[:, :width],
                    in1=tmp[:, :width], op=mybir.AluOpType.mult,
                )

            def store(ps2, width, m2, ns):
                o_sb = opool.tile([P, NBLK], f32, tag="o")
                nc.vector.tensor_copy(o_sb[:, :width], ps2[:, :width])
                nc.sync.dma_start(out=outT_r[:, m2, ns], in_=o_sb[:, :width])

            col = 0
            for nb, width in enumerate(blocks):
                ns = slice(col, col + width)
                col += width
                if nb == 0:
                    x_sb = x0_sb
                else:
                    x_sb = xpool.tile([P, KC, NBLK], cd, tag="x")
                    nc.gpsimd.dma_start(out=x_sb[:, :, :width],
                                        in_=xeT_r[:, :, ns])

                hid = hpool.tile([P, KH, NBLK], cd, tag="hid")
                if nb == 0:
                    # k-grouped: consume each weight chunk as its DMA lands,
                    # using 4 live PSUM accumulators per group.
                    for mg in range(0, KH, 4):
                        pss = [pspool.tile([P, NBLK], f32, tag="ps", name="ps")
                               for _ in range(4)]
                        for k in range(KC):
                            for j in range(4):
                                mm1(pss[j], x_sb, width, mg + j, k)
                        for j in range(4):
                            relu_sq(pss[j], hid, width, mg + j)
                    for mg2 in range(0, KC, 4):
                        pss = [pspool.tile([P, NBLK], f32, tag="ps", name="ps")
                               for _ in range(4)]
                        for k2 in range(KH):
                            for j in range(4):
                                mm2(pss[j], hid, width, mg2 + j, k2)
                        for j in range(4):
                            store(pss[j], width, mg2 + j, ns)
                else:
                    for m in range(KH):
                        ps = pspool.tile([P, NBLK], f32, tag="ps")
                        for k in range(KC):
                            mm1(ps, x_sb, width, m, k)
                        relu_sq(ps, hid, width, m)
                    for m2 in range(KC):
                        ps2 = pspool.tile([P, NBLK], f32, tag="ps")
                        for k2 in range(KH):
                            mm2(ps2, hid, width, m2, k2)
                        store(ps2, width, m2, ns)

    nc.compile()
    return nc


def _profile_hook():
    """NTFF capture via libaxon ctypes (used only when MOE_PROFILE_DIR is set)."""
    so_path = "/opt/axon/libaxon_pjrt.so"
    if not os.path.exists(so_path):
        return None
    lib = ctypes.CDLL(so_path)
    if not hasattr(lib, "axon_start_nrt_profile"):
        return None
    lib.axon_start_nrt_profile.argtypes = [
        ctypes.POINTER(ctypes.c_int64), ctypes.c_size_t,
    ]
    lib.axon_start_nrt_profile.restype = ctypes.c_int64
    lib.axon_stop_nrt_profile.argtypes = [ctypes.c_char_p]
    lib.axon_stop_nrt_profile.restype = ctypes.c_int64

    @contextlib.contextmanager
    def _hook(output_dir, device_ids):
        import jax
        jax.devices()
        ids = (ctypes.c_int64 * len(device_ids))(*device_ids)
        rc = lib.axon_start_nrt_profile(ids, len(device_ids))
        if rc != 0:
            raise RuntimeError(f"axon_start_nrt_profile rc={rc}")
        try:
            yield
        finally:
            n = lib.axon_stop_nrt_profile(str(output_dir).encode())
            print(f"profile: {n} file(s) written to {output_dir}")

    return _hook


def _run_device(xeT, fcwT_all, pjwT_all, blocks, mode):
    from concourse.bass_utils import run_bass_kernel_spmd

    key = (tuple(blocks), mode)
    if key not in _NC_CACHE:
        _NC_CACHE[key] = _build_nc(blocks, mode)
    nc = _NC_CACHE[key]

    in_maps = [
        {"xeT": xeT[e], "fcwT": fcwT_all[e], "pjwT": pjwT_all[e]}
        for e in range(E)
    ]
    core_ids = list(range(E))

    prof_dir = os.environ.get("MOE_PROFILE_DIR")
    if prof_dir:
        hook = _profile_hook()
        # warm-up run first so the profiled run measures a warm device
        run_bass_kernel_spmd(nc, in_maps, core_ids)
        with hook(prof_dir, core_ids):
            res = run_bass_kernel_spmd(nc, in_maps, core_ids)
    else:
        res = run_bass_kernel_spmd(nc, in_maps, core_ids)

    return np.stack([res.results[e]["outT"] for e in range(E)])


def kernel(x, router_w, fc_w, proj_w):
    x = np.asarray(x, np.float32)
    router_w = np.asarray(router_w, np.float32)
    fc_w = np.asarray(fc_w, np.float32)
    proj_w = np.asarray(proj_w, np.float32)

    mode = os.environ.get("MOE_DTYPE", "f32r")
    x_flat = x.reshape(N, C)

    # --- routing (mirrors the reference numerics in f32) ---
    scores = x_flat @ router_w.T                        # [N, E]
    mx = scores.max(-1, keepdims=True)
    ex = np.exp(scores - mx)
    probs = (ex / ex.sum(-1, keepdims=True)).astype(np.float32)
    top_idx = np.argsort(-probs, axis=-1, kind="stable")[:, :K]
    top_w = np.take_along_axis(probs, top_idx, -1)
    top_w = top_w / (top_w.sum(-1, keepdims=True) + 1e-10)

    fe = top_idx.reshape(-1)                            # [N*K]
    fw = top_w.reshape(-1).astype(np.float32)
    ft = np.repeat(np.arange(N), K)
    order = np.argsort(fe, kind="stable")
    se, st = fe[order], ft[order]
    counts = np.bincount(fe, minlength=E)
    offs = np.concatenate([[0], np.cumsum(counts)[:-1]]).astype(np.int64)
    pos_sorted = np.arange(N * K) - offs[se]
    kept = np.minimum(counts, CAP)

    blocks = _block_widths(kept.max())
    RT = sum(blocks)

    if mode == "bf16":
        import ml_dtypes
        io_np = ml_dtypes.bfloat16
    else:
        io_np = np.float32

    # --- pack per-expert token batches, transposed ---
    xeT = np.zeros((E, C, RT), io_np)
    for e in range(E):
        toks = st[offs[e]: offs[e] + kept[e]]
        xeT[e, :, :kept[e]] = x_flat[toks].T.astype(io_np)
    fcwT_all = np.ascontiguousarray(
        fc_w.transpose(0, 2, 1)).astype(io_np)    # [E, C, H]
    pjwT_all = np.ascontiguousarray(
        proj_w.transpose(0, 2, 1)).astype(io_np)  # [E, H, C]

    outT = _run_device(xeT, fcwT_all, pjwT_all, blocks, mode)  # [E, C, RT]

    # --- combine: weight each (token, slot) contribution and sum ---
    inv = np.empty(N * K, np.int64)
    inv[order] = np.arange(N * K)
    pos_flat = pos_sorted[inv]
    valid = pos_flat < CAP
    pos_c = np.where(valid, pos_flat, 0)
    w_eff = np.where(valid, fw, 0.0).astype(np.float32)

    gathered = outT[fe, :, pos_c]                              # [N*K, C]
    out = (gathered * w_eff[:, None]).reshape(N, K, C).sum(1)

    return (
        out.reshape(B, T, C).astype(np.float32),
        probs.reshape(B, T, E).astype(np.float32),
    )


# revision 7
# speedup vs baseline: 1.0267x; 1.0267x over previous
"""MoE layer (top-2 routing, 8 experts) on 8 TRN2 NeuronCores.

Strategy: expert-parallel. The host computes routing (router matmul, softmax,
top-2, capacity dispatch — cheap integer/index work) and packs each expert's
tokens densely. Core e runs expert e's MLP (fc -> relu^2 -> proj) over its
padded token batch with fp32r matmuls. The host then gathers, weights and
combines the expert outputs.

All tensors are laid out transposed ([feature, token]) so the device kernel
needs no on-chip transposes: both matmuls contract over the partition dim.
"""
import contextlib
import ctypes
import os

import numpy as np

B, T, C, E, H, K = 4, 2048, 1024, 8, 2048, 2
N = B * T
CAP = 2 * N * K // E  # per-expert capacity; overflow tokens are dropped
NBLK = 512            # token block (matmul moving free dim)
P = 128

_NC_CACHE = {}


def _block_widths(max_kept):
    """Token-block widths covering max_kept: full 512s plus a 256 tail if it fits."""
    full, rem = divmod(max(int(max_kept), 1), NBLK)
    if rem == 0:
        return [NBLK] * full
    if rem <= NBLK // 2:
        return [NBLK] * full + [NBLK // 2]
    return [NBLK] * (full + 1)


def _build_nc(blocks, mode):
    from concourse import bacc, mybir, tile

    f32 = mybir.dt.float32
    cd = {"f32r": mybir.dt.float32r, "bf16": mybir.dt.bfloat16,
          "f32": mybir.dt.float32}[mode]
    io_dt = f32 if mode in ("f32r", "f32") else mybir.dt.bfloat16
    KC = C // P   # k-tiles for fc (contract over C)
    KH = H // P   # k-tiles for proj (contract over H)
    RT = sum(blocks)

    def rcast(ap):
        return ap.bitcast(cd) if mode == "f32r" else ap

    nc = bacc.Bacc("TRN2", target_bir_lowering=False, debug=False)
    xeT = nc.dram_tensor("xeT", [C, RT], io_dt, kind="ExternalInput")
    fcwT = nc.dram_tensor("fcwT", [C, H], io_dt, kind="ExternalInput")
    pjwT = nc.dram_tensor("pjwT", [H, C], io_dt, kind="ExternalInput")
    outT = nc.dram_tensor("outT", [C, RT], f32, kind="ExternalOutput")

    xeT_r = rcast(xeT[:].rearrange("(k p) n -> p k n", p=P))
    fcw_r = rcast(fcwT[:].rearrange("(k p) h -> p k h", p=P))
    pjw_r = rcast(pjwT[:].rearrange("(k p) c -> p k c", p=P))
    outT_r = outT[:].rearrange("(m p) n -> p m n", p=P)

    with tile.TileContext(nc) as tc:
        with (
            tc.tile_pool(name="wpool", bufs=1) as wpool,
            tc.tile_pool(name="xpool", bufs=2) as xpool,
            tc.tile_pool(name="hpool", bufs=1) as hpool,
            tc.tile_pool(name="tpool", bufs=3) as tpool,
            tc.tile_pool(name="opool", bufs=3) as opool,
            tc.tile_pool(name="pspool", bufs=8, space="PSUM") as pspool,
        ):
            fcw_sb = wpool.tile([P, KC, H], cd)
            pjw_sb = wpool.tile([P, KH, C], cd)
            x0_sb = xpool.tile([P, KC, NBLK], cd, tag="x")
            w0 = blocks[0]
            dma_q = [nc.sync, nc.scalar]
            # Issue order sets priority within each HWDGE queue: block-0
            # activations first (small), then fc weights, then proj weights.
            # Chunks alternate between the two queues to use both DMA paths.
            for k in range(KC):
                dma_q[k % 2].dma_start(out=x0_sb[:, k, :w0],
                                       in_=xeT_r[:, k, 0:w0])
            for k in range(KC):
                dma_q[k % 2].dma_start(out=fcw_sb[:, k, :], in_=fcw_r[:, k, :])
            # proj weights go through gpsimd's SWDGE queues so the scalar
            # engine (which runs the relus) never blocks on DMA queue slots
            for k in range(0, KH, 2):
                nc.gpsimd.dma_start(out=pjw_sb[:, k:k + 2, :],
                                    in_=pjw_r[:, k:k + 2, :])

            def mm1(ps, x_sb, width, m, k):
                nc.tensor.matmul(
                    ps[:, :width],
                    lhsT=fcw_sb[:, k, m * P:(m + 1) * P],
                    rhs=x_sb[:, k, :width],
                    start=(k == 0), stop=(k == KC - 1),
                )

            def mm2(ps, hid, width, m2, k2):
                nc.tensor.matmul(
                    ps[:, :width],
                    lhsT=pjw_sb[:, k2, m2 * P:(m2 + 1) * P],
                    rhs=hid[:, k2, :width],
                    start=(k2 == 0), stop=(k2 == KH - 1),
                )

            def relu_sq(ps, hid, width, m):
                tmp = tpool.tile([P, NBLK], f32, tag="tmp")
                nc.scalar.activation(
                    tmp[:, :width], ps[:, :width],
                    mybir.ActivationFunctionType.Relu,
                )
                nc.vector.tensor_tensor(
                    out=hid[:, m, :width], in0=tmp[:, :width],
                    in1=tmp[:, :width], op=mybir.AluOpType.mult,
                )

            def store(ps2, width, m2, ns):
                o_sb = opool.tile([P, NBLK], f32, tag="o")
                nc.vector.tensor_copy(o_sb[:, :width], ps2[:, :width])
                nc.sync.dma_start(out=outT_r[:, m2, ns], in_=o_sb[:, :width])

            col = 0
            for nb, width in enumerate(blocks):
                ns = slice(col, col + width)
                col += width
                if nb == 0:
                    x_sb = x0_sb
                else:
                    x_sb = xpool.tile([P, KC, NBLK], cd, tag="x")
                    nc.gpsimd.dma_start(out=x_sb[:, :, :width],
                                        in_=xeT_r[:, :, ns])

                hid = hpool.tile([P, KH, NBLK], cd, tag="hid")
                if nb == 0:
                    # k-grouped: consume each weight chunk as its DMA lands,
                    # using 4 live PSUM accumulators per group.
                    for mg in range(0, KH, 4):
                        pss = [pspool.tile([P, NBLK], f32, tag="ps", name="ps")
                               for _ in range(4)]
                        for k in range(KC):
                            for j in range(4):
                                mm1(pss[j], x_sb, width, mg + j, k)
                        for j in range(4):
                            relu_sq(pss[j], hid, width, mg + j)
                    for mg2 in range(0, KC, 4):
                        pss = [pspool.tile([P, NBLK], f32, tag="ps", name="ps")
                               for _ in range(4)]
                        for k2 in range(KH):
                            for j in range(4):
                                mm2(pss[j], hid, width, mg2 + j, k2)
                        for j in range(4):
                            store(pss[j], width, mg2 + j, ns)
                else:
                    for m in range(KH):
                        ps = pspool.tile([P, NBLK], f32, tag="ps")
                        for k in range(KC):
                            mm1(ps, x_sb, width, m, k)
                        relu_sq(ps, hid, width, m)
                    for m2 in range(KC):
                        ps2 = pspool.tile([P, NBLK], f32, tag="ps")
                        for k2 in range(KH):
                            mm2(ps2, hid, width, m2, k2)
                        store(ps2, width, m2, ns)

    nc.compile()
    return nc


def _profile_hook():
    """NTFF capture via libaxon ctypes (used only when MOE_PROFILE_DIR is set)."""
    so_path = "/opt/axon/libaxon_pjrt.so"
    if not os.path.exists(so_path):
        return None
    lib = ctypes.CDLL(so_path)
    if not hasattr(lib, "axon_start_nrt_profile"):
        return None
    lib.axon_start_nrt_profile.argtypes = [
        ctypes.POINTER(ctypes.c_int64), ctypes.c_size_t,
    ]
    lib.axon_start_nrt_profile.restype = ctypes.c_int64
    lib.axon_stop_nrt_profile.argtypes = [ctypes.c_char_p]
    lib.axon_stop_nrt_profile.restype = ctypes.c_int64

    @contextlib.contextmanager
    def _hook(output_dir, device_ids):
        import jax
        jax.devices()
        ids = (ctypes.c_int64 * len(device_ids))(*device_ids)
        rc = lib.axon_start_nrt_profile(ids, len(device_ids))
        if rc != 0:
            raise RuntimeError(f"axon_start_nrt_profile rc={rc}")
        try:
            yield
        finally:
            n = lib.axon_stop_nrt_profile(str(output_dir).encode())
            print(f"profile: {n} file(s) written to {output_dir}")

    return _hook


def _run_device(xeT, fcwT_all, pjwT_all, blocks, mode):
    from concourse.bass_utils import run_bass_kernel_spmd

    key = (tuple(blocks), mode)
    if key not in _NC_CACHE:
        _NC_CACHE[key] = _build_nc(blocks, mode)
    nc = _NC_CACHE[key]

    in_maps = [
        {"xeT": xeT[e], "fcwT": fcwT_all[e], "pjwT": pjwT_all[e]}
        for e in range(E)
    ]
    core_ids = list(range(E))

    prof_dir = os.environ.get("MOE_PROFILE_DIR")
    if prof_dir:
        hook = _profile_hook()
        # warm-up run first so the profiled run measures a warm device
        run_bass_kernel_spmd(nc, in_maps, core_ids)
        with hook(prof_dir, core_ids):
            res = run_bass_kernel_spmd(nc, in_maps, core_ids)
    else:
        res = run_bass_kernel_spmd(nc, in_maps, core_ids)

    return np.stack([res.results[e]["outT"] for e in range(E)])


def kernel(x, router_w, fc_w, proj_w):
    x = np.asarray(x, np.float32)
    router_w = np.asarray(router_w, np.float32)
    fc_w = np.asarray(fc_w, np.float32)
    proj_w = np.asarray(proj_w, np.float32)

    mode = os.environ.get("MOE_DTYPE", "f32r")
    x_flat = x.reshape(N, C)

    # --- routing (mirrors the reference numerics in f32) ---
    scores = x_flat @ router_w.T                        # [N, E]
    mx = scores.max(-1, keepdims=True)
    ex = np.exp(scores - mx)
    probs = (ex / ex.sum(-1, keepdims=True)).astype(np.float32)
    top_idx = np.argsort(-probs, axis=-1, kind="stable")[:, :K]
    top_w = np.take_along_axis(probs, top_idx, -1)
    top_w = top_w / (top_w.sum(-1, keepdims=True) + 1e-10)

    fe = top_idx.reshape(-1)                            # [N*K]
    fw = top_w.reshape(-1).astype(np.float32)
    ft = np.repeat(np.arange(N), K)
    order = np.argsort(fe, kind="stable")
    se, st = fe[order], ft[order]
    counts = np.bincount(fe, minlength=E)
    offs = np.concatenate([[0], np.cumsum(counts)[:-1]]).astype(np.int64)
    pos_sorted = np.arange(N * K) - offs[se]
    kept = np.minimum(counts, CAP)

    blocks = _block_widths(kept.max())
    RT = sum(blocks)

    if mode == "bf16":
        import ml_dtypes
        io_np = ml_dtypes.bfloat16
    else:
        io_np = np.float32

    # --- pack per-expert token batches, transposed ---
    xeT = np.zeros((E, C, RT), io_np)
    for e in range(E):
        toks = st[offs[e]: offs[e] + kept[e]]
        xeT[e, :, :kept[e]] = x_flat[toks].T.astype(io_np)
    fcwT_all = np.ascontiguousarray(
        fc_w.transpose(0, 2, 1)).astype(io_np)    # [E, C, H]
    pjwT_all = np.ascontiguousarray(
        proj_w.transpose(0, 2, 1)).astype(io_np)  # [E, H, C]

    outT = _run_device(xeT, fcwT_all, pjwT_all, blocks, mode)  # [E, C, RT]

    # --- combine: weight each (token, slot) contribution and sum ---
    inv = np.empty(N * K, np.int64)
    inv[order] = np.arange(N * K)
    pos_flat = pos_sorted[inv]
    valid = pos_flat < CAP
    pos_c = np.where(valid, pos_flat, 0)
    w_eff = np.where(valid, fw, 0.0).astype(np.float32)

    gathered = outT[fe, :, pos_c]                              # [N*K, C]
    out = (gathered * w_eff[:, None]).reshape(N, K, C).sum(1)

    return (
        out.reshape(B, T, C).astype(np.float32),
        probs.reshape(B, T, E).astype(np.float32),
    )


# revision 9
# speedup vs baseline: 1.1367x; 1.1072x over previous
"""MoE layer (top-2 routing, 8 experts) on 8 TRN2 NeuronCores.

Strategy: expert-parallel. The host computes routing (router matmul, softmax,
top-2, capacity dispatch — cheap integer/index work) and packs each expert's
tokens densely. Core e runs expert e's MLP (fc -> relu^2 -> proj) over its
padded token batch with fp32r matmuls. The host then gathers, weights and
combines the expert outputs.

All tensors are laid out transposed ([feature, token]) so the device kernel
needs no on-chip transposes: both matmuls contract over the partition dim.
"""
import contextlib
import ctypes
import os

import numpy as np

B, T, C, E, H, K = 4, 2048, 1024, 8, 2048, 2
N = B * T
CAP = 2 * N * K // E  # per-expert capacity; overflow tokens are dropped
NBLK = 512            # token block (matmul moving free dim)
P = 128

_NC_CACHE = {}


def _block_widths(max_kept):
    """Token-block widths covering max_kept: full 512s plus a 256 tail if it fits."""
    full, rem = divmod(max(int(max_kept), 1), NBLK)
    if rem == 0:
        return [NBLK] * full
    if rem <= NBLK // 2:
        return [NBLK] * full + [NBLK // 2]
    return [NBLK] * (full + 1)


def _build_nc(blocks, mode):
    from concourse import bacc, mybir, tile

    f32 = mybir.dt.float32
    cd = {"f32r": mybir.dt.float32r, "bf16": mybir.dt.bfloat16,
          "f16": mybir.dt.float16, "f32": mybir.dt.float32}[mode]
    io_dt = f32 if mode in ("f32r", "f32") else cd
    KC = C // P   # k-tiles for fc (contract over C)
    KH = H // P   # k-tiles for proj (contract over H)
    RT = sum(blocks)

    def rcast(ap):
        return ap.bitcast(cd) if mode == "f32r" else ap

    nc = bacc.Bacc("TRN2", target_bir_lowering=False, debug=False)
    xeT = nc.dram_tensor("xeT", [C, RT], io_dt, kind="ExternalInput")
    fcwT = nc.dram_tensor("fcwT", [C, H], io_dt, kind="ExternalInput")
    pjwT = nc.dram_tensor("pjwT", [H, C], io_dt, kind="ExternalInput")
    outT = nc.dram_tensor("outT", [C, RT], f32, kind="ExternalOutput")

    xeT_r = rcast(xeT[:].rearrange("(k p) n -> p k n", p=P))
    fcw_r = rcast(fcwT[:].rearrange("(k p) h -> p k h", p=P))
    pjw_r = rcast(pjwT[:].rearrange("(k p) c -> p k c", p=P))
    outT_r = outT[:].rearrange("(m p) n -> p m n", p=P)

    with tile.TileContext(nc) as tc:
        with (
            tc.tile_pool(name="wpool", bufs=1) as wpool,
            tc.tile_pool(name="xpool", bufs=2) as xpool,
            tc.tile_pool(name="hpool", bufs=1) as hpool,
            tc.tile_pool(name="tpool", bufs=3) as tpool,
            tc.tile_pool(name="opool", bufs=3) as opool,
            tc.tile_pool(name="pspool", bufs=8, space="PSUM") as pspool,
        ):
            fcw_sb = wpool.tile([P, KC, H], cd)
            pjw_sb = wpool.tile([P, KH, C], cd)
            x0_sb = xpool.tile([P, KC, NBLK], cd, tag="x")
            w0 = blocks[0]
            dma_q = [nc.sync, nc.scalar]
            # Issue order sets priority within each HWDGE queue: block-0
            # activations first (small), then fc weights, then proj weights.
            # Chunks alternate between the two queues to use both DMA paths.
            for k in range(KC):
                dma_q[k % 2].dma_start(out=x0_sb[:, k, :w0],
                                       in_=xeT_r[:, k, 0:w0])
            for k in range(KC):
                dma_q[k % 2].dma_start(out=fcw_sb[:, k, :], in_=fcw_r[:, k, :])
            # proj weights go through gpsimd's SWDGE queues so the scalar
            # engine (which runs the relus) never blocks on DMA queue slots
            for k in range(0, KH, 2):
                nc.gpsimd.dma_start(out=pjw_sb[:, k:k + 2, :],
                                    in_=pjw_r[:, k:k + 2, :])

            def mm1(ps, x_sb, width, m, k):
                nc.tensor.matmul(
                    ps[:, :width],
                    lhsT=fcw_sb[:, k, m * P:(m + 1) * P],
                    rhs=x_sb[:, k, :width],
                    start=(k == 0), stop=(k == KC - 1),
                )

            def mm2(ps, hid, width, m2, k2):
                nc.tensor.matmul(
                    ps[:, :width],
                    lhsT=pjw_sb[:, k2, m2 * P:(m2 + 1) * P],
                    rhs=hid[:, k2, :width],
                    start=(k2 == 0), stop=(k2 == KH - 1),
                )

            def relu_sq(ps, hid, width, m):
                tmp = tpool.tile([P, NBLK], f32, tag="tmp")
                nc.scalar.activation(
                    tmp[:, :width], ps[:, :width],
                    mybir.ActivationFunctionType.Relu,
                )
                nc.vector.tensor_tensor(
                    out=hid[:, m, :width], in0=tmp[:, :width],
                    in1=tmp[:, :width], op=mybir.AluOpType.mult,
                )

            def store(ps2, width, m2, ns):
                o_sb = opool.tile([P, NBLK], f32, tag="o")
                nc.vector.tensor_copy(o_sb[:, :width], ps2[:, :width])
                nc.sync.dma_start(out=outT_r[:, m2, ns], in_=o_sb[:, :width])

            col = 0
            for nb, width in enumerate(blocks):
                ns = slice(col, col + width)
                col += width
                if nb == 0:
                    x_sb = x0_sb
                else:
                    x_sb = xpool.tile([P, KC, NBLK], cd, tag="x")
                    nc.gpsimd.dma_start(out=x_sb[:, :, :width],
                                        in_=xeT_r[:, :, ns])

                hid = hpool.tile([P, KH, NBLK], cd, tag="hid")
                if nb == 0:
                    # k-grouped: consume each weight chunk as its DMA lands,
                    # using 4 live PSUM accumulators per group.
                    for mg in range(0, KH, 4):
                        pss = [pspool.tile([P, NBLK], f32, tag="ps", name="ps")
                               for _ in range(4)]
                        for k in range(KC):
                            for j in range(4):
                                mm1(pss[j], x_sb, width, mg + j, k)
                        for j in range(4):
                            relu_sq(pss[j], hid, width, mg + j)
                    for mg2 in range(0, KC, 4):
                        pss = [pspool.tile([P, NBLK], f32, tag="ps", name="ps")
                               for _ in range(4)]
                        for k2 in range(KH):
                            for j in range(4):
                                mm2(pss[j], hid, width, mg2 + j, k2)
                        for j in range(4):
                            store(pss[j], width, mg2 + j, ns)
                else:
                    for m in range(KH):
                        ps = pspool.tile([P, NBLK], f32, tag="ps")
                        for k in range(KC):
                            mm1(ps, x_sb, width, m, k)
                        relu_sq(ps, hid, width, m)
                    for m2 in range(KC):
                        ps2 = pspool.tile([P, NBLK], f32, tag="ps")
                        for k2 in range(KH):
                            mm2(ps2, hid, width, m2, k2)
                        store(ps2, width, m2, ns)

    nc.compile()
    return nc


def _profile_hook():
    """NTFF capture via libaxon ctypes (used only when MOE_PROFILE_DIR is set)."""
    so_path = "/opt/axon/libaxon_pjrt.so"
    if not os.path.exists(so_path):
        return None
    lib = ctypes.CDLL(so_path)
    if not hasattr(lib, "axon_start_nrt_profile"):
        return None
    lib.axon_start_nrt_profile.argtypes = [
        ctypes.POINTER(ctypes.c_int64), ctypes.c_size_t,
    ]
    lib.axon_start_nrt_profile.restype = ctypes.c_int64
    lib.axon_stop_nrt_profile.argtypes = [ctypes.c_char_p]
    lib.axon_stop_nrt_profile.restype = ctypes.c_int64

    @contextlib.contextmanager
    def _hook(output_dir, device_ids):
        import jax
        jax.devices()
        ids = (ctypes.c_int64 * len(device_ids))(*device_ids)
        rc = lib.axon_start_nrt_profile(ids, len(device_ids))
        if rc != 0:
            raise RuntimeError(f"axon_start_nrt_profile rc={rc}")
        try:
            yield
        finally:
            n = lib.axon_stop_nrt_profile(str(output_dir).encode())
            print(f"profile: {n} file(s) written to {output_dir}")

    return _hook


def _run_device(xeT, fcwT_all, pjwT_all, blocks, mode):
    from concourse.bass_utils import run_bass_kernel_spmd

    key = (tuple(blocks), mode)
    if key not in _NC_CACHE:
        _NC_CACHE[key] = _build_nc(blocks, mode)
    nc = _NC_CACHE[key]

    in_maps = [
        {"xeT": xeT[e], "fcwT": fcwT_all[e], "pjwT": pjwT_all[e]}
        for e in range(E)
    ]
    core_ids = list(range(E))

    prof_dir = os.environ.get("MOE_PROFILE_DIR")
    if prof_dir:
        hook = _profile_hook()
        # warm-up run first so the profiled run measures a warm device
        run_bass_kernel_spmd(nc, in_maps, core_ids)
        with hook(prof_dir, core_ids):
            res = run_bass_kernel_spmd(nc, in_maps, core_ids)
    else:
        res = run_bass_kernel_spmd(nc, in_maps, core_ids)

    return np.stack([res.results[e]["outT"] for e in range(E)])


def kernel(x, router_w, fc_w, proj_w):
    x = np.asarray(x, np.float32)
    router_w = np.asarray(router_w, np.float32)
    fc_w = np.asarray(fc_w, np.float32)
    proj_w = np.asarray(proj_w, np.float32)

    mode = os.environ.get("MOE_DTYPE", "f32r")
    x_flat = x.reshape(N, C)

    # --- routing (mirrors the reference numerics in f32) ---
    scores = x_flat @ router_w.T                        # [N, E]
    mx = scores.max(-1, keepdims=True)
    ex = np.exp(scores - mx)
    probs = (ex / ex.sum(-1, keepdims=True)).astype(np.float32)
    top_idx = np.argsort(-probs, axis=-1, kind="stable")[:, :K]
    top_w = np.take_along_axis(probs, top_idx, -1)
    top_w = top_w / (top_w.sum(-1, keepdims=True) + 1e-10)

    fe = top_idx.reshape(-1)                            # [N*K]
    fw = top_w.reshape(-1).astype(np.float32)
    ft = np.repeat(np.arange(N), K)
    order = np.argsort(fe, kind="stable")
    se, st = fe[order], ft[order]
    counts = np.bincount(fe, minlength=E)
    offs = np.concatenate([[0], np.cumsum(counts)[:-1]]).astype(np.int64)
    pos_sorted = np.arange(N * K) - offs[se]
    kept = np.minimum(counts, CAP)

    blocks = _block_widths(kept.max())
    RT = sum(blocks)

    if mode == "bf16":
        import ml_dtypes
        io_np = ml_dtypes.bfloat16
    elif mode == "f16":
        io_np = np.float16
    else:
        io_np = np.float32

    # --- pack per-expert token batches, transposed ---
    xeT = np.zeros((E, C, RT), io_np)
    for e in range(E):
        toks = st[offs[e]: offs[e] + kept[e]]
        xeT[e, :, :kept[e]] = x_flat[toks].T.astype(io_np)
    fcwT_all = np.ascontiguousarray(
        fc_w.transpose(0, 2, 1)).astype(io_np)    # [E, C, H]
    pjwT_all = np.ascontiguousarray(
        proj_w.transpose(0, 2, 1)).astype(io_np)  # [E, H, C]

    outT = _run_device(xeT, fcwT_all, pjwT_all, blocks, mode)  # [E, C, RT]

    # --- combine: weight each (token, slot) contribution and sum ---
    inv = np.empty(N * K, np.int64)
    inv[order] = np.arange(N * K)
    pos_flat = pos_sorted[inv]
    valid = pos_flat < CAP
    pos_c = np.where(valid, pos_flat, 0)
    w_eff = np.where(valid, fw, 0.0).astype(np.float32)

    gathered = outT[fe, :, pos_c]                              # [N*K, C]
    out = (gathered * w_eff[:, None]).reshape(N, K, C).sum(1)

    return (
        out.reshape(B, T, C).astype(np.float32),
        probs.reshape(B, T, E).astype(np.float32),
    )


# revision 12
# speedup vs baseline: 1.1484x; 1.0102x over previous
"""MoE layer (top-2 routing, 8 experts) on 8 TRN2 NeuronCores.

Strategy: expert-parallel. The host computes routing (router matmul, softmax,
top-2, capacity dispatch — cheap integer/index work) and packs each expert's
tokens densely. Core e runs expert e's MLP (fc -> relu^2 -> proj) over its
padded token batch with fp32r matmuls. The host then gathers, weights and
combines the expert outputs.

All tensors are laid out transposed ([feature, token]) so the device kernel
needs no on-chip transposes: both matmuls contract over the partition dim.
"""
import contextlib
import ctypes
import os

import numpy as np

B, T, C, E, H, K = 4, 2048, 1024, 8, 2048, 2
N = B * T
CAP = 2 * N * K // E  # per-expert capacity; overflow tokens are dropped
NBLK = 512            # token block (matmul moving free dim)
P = 128

_NC_CACHE = {}


def _block_widths(max_kept):
    """Token-block widths covering max_kept: full 512s plus a 256 tail if it fits."""
    full, rem = divmod(max(int(max_kept), 1), NBLK)
    if rem == 0:
        return [NBLK] * full
    if rem <= NBLK // 2:
        return [NBLK] * full + [NBLK // 2]
    return [NBLK] * (full + 1)


def _build_nc(blocks, mode):
    from concourse import bacc, mybir, tile

    f32 = mybir.dt.float32
    cd = {"f32r": mybir.dt.float32r, "bf16": mybir.dt.bfloat16,
          "f16": mybir.dt.float16, "f32": mybir.dt.float32}[mode]
    io_dt = f32 if mode in ("f32r", "f32") else cd
    KC = C // P   # k-tiles for fc (contract over C)
    KH = H // P   # k-tiles for proj (contract over H)
    RT = sum(blocks)

    def rcast(ap):
        return ap.bitcast(cd) if mode == "f32r" else ap

    nc = bacc.Bacc("TRN2", target_bir_lowering=False, debug=False)
    xeT = nc.dram_tensor("xeT", [C, RT], io_dt, kind="ExternalInput")
    fcwT = nc.dram_tensor("fcwT", [C, H], io_dt, kind="ExternalInput")
    pjwT = nc.dram_tensor("pjwT", [H, C], io_dt, kind="ExternalInput")
    outT = nc.dram_tensor("outT", [C, RT], f32, kind="ExternalOutput")

    xeT_r = rcast(xeT[:].rearrange("(k p) n -> p k n", p=P))
    fcw_r = rcast(fcwT[:].rearrange("(k p) h -> p k h", p=P))
    pjw_r = rcast(pjwT[:].rearrange("(k p) c -> p k c", p=P))
    outT_r = outT[:].rearrange("(m p) n -> p m n", p=P)

    with tile.TileContext(nc) as tc:
        with (
            tc.tile_pool(name="wpool", bufs=1) as wpool,
            tc.tile_pool(name="xpool", bufs=2) as xpool,
            tc.tile_pool(name="hpool", bufs=1) as hpool,
            tc.tile_pool(name="tpool", bufs=3) as tpool,
            tc.tile_pool(name="opool", bufs=3) as opool,
            tc.tile_pool(name="pspool", bufs=8, space="PSUM") as pspool,
        ):
            fcw_sb = wpool.tile([P, KC, H], cd)
            pjw_sb = wpool.tile([P, KH, C], cd)
            x0_sb = xpool.tile([P, KC, NBLK], cd, tag="x")
            w0 = blocks[0]
            # Two parallel HWDGE streams with strict priority by queue order:
            #   scalar queue: x block 0, then proj weights (needed ~30us in)
            #   sync queue:   fc weight chunks (the first matmul's input)
            # The scalar engine stalls on DMA queue slots while issuing these,
            # so block 0 keeps its relus off the scalar engine (DVE instead).
            nc.scalar.dma_start(out=x0_sb[:, :, :w0], in_=xeT_r[:, :, 0:w0])
            for k in range(KC):
                nc.sync.dma_start(out=fcw_sb[:, k, :], in_=fcw_r[:, k, :])
            for k in range(0, KH, 4):
                nc.scalar.dma_start(out=pjw_sb[:, k:k + 4, :],
                                    in_=pjw_r[:, k:k + 4, :])

            def mm1(ps, x_sb, width, m, k):
                nc.tensor.matmul(
                    ps[:, :width],
                    lhsT=fcw_sb[:, k, m * P:(m + 1) * P],
                    rhs=x_sb[:, k, :width],
                    start=(k == 0), stop=(k == KC - 1),
                )

            def mm2(ps, hid, width, m2, k2):
                nc.tensor.matmul(
                    ps[:, :width],
                    lhsT=pjw_sb[:, k2, m2 * P:(m2 + 1) * P],
                    rhs=hid[:, k2, :width],
                    start=(k2 == 0), stop=(k2 == KH - 1),
                )

            def relu_sq(ps, hid, width, m, on_dve=False):
                tmp = tpool.tile([P, NBLK], f32, tag="tmp")
                if on_dve:
                    nc.vector.tensor_scalar(
                        tmp[:, :width], ps[:, :width], 0.0, scalar2=None,
                        op0=mybir.AluOpType.max,
                    )
                else:
                    nc.scalar.activation(
                        tmp[:, :width], ps[:, :width],
                        mybir.ActivationFunctionType.Relu,
                    )
                nc.vector.tensor_tensor(
                    out=hid[:, m, :width], in0=tmp[:, :width],
                    in1=tmp[:, :width], op=mybir.AluOpType.mult,
                )

            def store(ps2, width, m2, ns):
                o_sb = opool.tile([P, NBLK], f32, tag="o")
                nc.vector.tensor_copy(o_sb[:, :width], ps2[:, :width])
                nc.sync.dma_start(out=outT_r[:, m2, ns], in_=o_sb[:, :width])

            col = 0
            for nb, width in enumerate(blocks):
                ns = slice(col, col + width)
                col += width
                if nb == 0:
                    x_sb = x0_sb
                else:
                    x_sb = xpool.tile([P, KC, NBLK], cd, tag="x")
                    nc.gpsimd.dma_start(out=x_sb[:, :, :width],
                                        in_=xeT_r[:, :, ns])

                hid = hpool.tile([P, KH, NBLK], cd, tag="hid")
                if nb == 0:
                    # k-grouped: consume each weight chunk as its DMA lands,
                    # using 4 live PSUM accumulators per group.
                    for mg in range(0, KH, 4):
                        pss = [pspool.tile([P, NBLK], f32, tag="ps", name="ps")
                               for _ in range(4)]
                        for k in range(KC):
                            for j in range(4):
                                mm1(pss[j], x_sb, width, mg + j, k)
                        for j in range(4):
                            relu_sq(pss[j], hid, width, mg + j, on_dve=True)
                    for mg2 in range(0, KC, 4):
                        pss = [pspool.tile([P, NBLK], f32, tag="ps", name="ps")
                               for _ in range(4)]
                        for k2 in range(KH):
                            for j in range(4):
                                mm2(pss[j], hid, width, mg2 + j, k2)
                        for j in range(4):
                            store(pss[j], width, mg2 + j, ns)
                else:
                    for m in range(KH):
                        ps = pspool.tile([P, NBLK], f32, tag="ps")
                        for k in range(KC):
                            mm1(ps, x_sb, width, m, k)
                        relu_sq(ps, hid, width, m)
                    for m2 in range(KC):
                        ps2 = pspool.tile([P, NBLK], f32, tag="ps")
                        for k2 in range(KH):
                            mm2(ps2, hid, width, m2, k2)
                        store(ps2, width, m2, ns)

    nc.compile()
    return nc


def _profile_hook():
    """NTFF capture via libaxon ctypes (used only when MOE_PROFILE_DIR is set)."""
    so_path = "/opt/axon/libaxon_pjrt.so"
    if not os.path.exists(so_path):
        return None
    lib = ctypes.CDLL(so_path)
    if not hasattr(lib, "axon_start_nrt_profile"):
        return None
    lib.axon_start_nrt_profile.argtypes = [
        ctypes.POINTER(ctypes.c_int64), ctypes.c_size_t,
    ]
    lib.axon_start_nrt_profile.restype = ctypes.c_int64
    lib.axon_stop_nrt_profile.argtypes = [ctypes.c_char_p]
    lib.axon_stop_nrt_profile.restype = ctypes.c_int64

    @contextlib.contextmanager
    def _hook(output_dir, device_ids):
        import jax
        jax.devices()
        ids = (ctypes.c_int64 * len(device_ids))(*device_ids)
        rc = lib.axon_start_nrt_profile(ids, len(device_ids))
        if rc != 0:
            raise RuntimeError(f"axon_start_nrt_profile rc={rc}")
        try:
            yield
        finally:
            n = lib.axon_stop_nrt_profile(str(output_dir).encode())
            print(f"profile: {n} file(s) written to {output_dir}")

    return _hook


def _run_device(xeT, fcwT_all, pjwT_all, blocks, mode):
    from concourse.bass_utils import run_bass_kernel_spmd

    key = (tuple(blocks), mode)
    if key not in _NC_CACHE:
        _NC_CACHE[key] = _build_nc(blocks, mode)
    nc = _NC_CACHE[key]

    in_maps = [
        {"xeT": xeT[e], "fcwT": fcwT_all[e], "pjwT": pjwT_all[e]}
        for e in range(E)
    ]
    core_ids = list(range(E))

    prof_dir = os.environ.get("MOE_PROFILE_DIR")
    if prof_dir:
        hook = _profile_hook()
        # warm-up run first so the profiled run measures a warm device
        run_bass_kernel_spmd(nc, in_maps, core_ids)
        with hook(prof_dir, core_ids):
            res = run_bass_kernel_spmd(nc, in_maps, core_ids)
    else:
        res = run_bass_kernel_spmd(nc, in_maps, core_ids)

    return np.stack([res.results[e]["outT"] for e in range(E)])


def kernel(x, router_w, fc_w, proj_w):
    x = np.asarray(x, np.float32)
    router_w = np.asarray(router_w, np.float32)
    fc_w = np.asarray(fc_w, np.float32)
    proj_w = np.asarray(proj_w, np.float32)

    mode = os.environ.get("MOE_DTYPE", "f32r")
    x_flat = x.reshape(N, C)

    # --- routing (mirrors the reference numerics in f32) ---
    scores = x_flat @ router_w.T                        # [N, E]
    mx = scores.max(-1, keepdims=True)
    ex = np.exp(scores - mx)
    probs = (ex / ex.sum(-1, keepdims=True)).astype(np.float32)
    top_idx = np.argsort(-probs, axis=-1, kind="stable")[:, :K]
    top_w = np.take_along_axis(probs, top_idx, -1)
    top_w = top_w / (top_w.sum(-1, keepdims=True) + 1e-10)

    fe = top_idx.reshape(-1)                            # [N*K]
    fw = top_w.reshape(-1).astype(np.float32)
    ft = np.repeat(np.arange(N), K)
    order = np.argsort(fe, kind="stable")
    se, st = fe[order], ft[order]
    counts = np.bincount(fe, minlength=E)
    offs = np.concatenate([[0], np.cumsum(counts)[:-1]]).astype(np.int64)
    pos_sorted = np.arange(N * K) - offs[se]
    kept = np.minimum(counts, CAP)

    blocks = _block_widths(kept.max())
    RT = sum(blocks)

    if mode == "bf16":
        import ml_dtypes
        io_np = ml_dtypes.bfloat16
    elif mode == "f16":
        io_np = np.float16
    else:
        io_np = np.float32

    # --- pack per-expert token batches, transposed ---
    xeT = np.zeros((E, C, RT), io_np)
    for e in range(E):
        toks = st[offs[e]: offs[e] + kept[e]]
        xeT[e, :, :kept[e]] = x_flat[toks].T.astype(io_np)
    fcwT_all = np.ascontiguousarray(
        fc_w.transpose(0, 2, 1)).astype(io_np)    # [E, C, H]
    pjwT_all = np.ascontiguousarray(
        proj_w.transpose(0, 2, 1)).astype(io_np)  # [E, H, C]

    outT = _run_device(xeT, fcwT_all, pjwT_all, blocks, mode)  # [E, C, RT]

    # --- combine: weight each (token, slot) contribution and sum ---
    inv = np.empty(N * K, np.int64)
    inv[order] = np.arange(N * K)
    pos_flat = pos_sorted[inv]
    valid = pos_flat < CAP
    pos_c = np.where(valid, pos_flat, 0)
    w_eff = np.where(valid, fw, 0.0).astype(np.float32)

    gathered = outT[fe, :, pos_c]                              # [N*K, C]
    out = (gathered * w_eff[:, None]).reshape(N, K, C).sum(1)

    return (
        out.reshape(B, T, C).astype(np.float32),
        probs.reshape(B, T, E).astype(np.float32),
    )


# revision 15
# speedup vs baseline: 1.1627x; 1.0125x over previous
"""MoE layer (top-2 routing, 8 experts) on 8 TRN2 NeuronCores.

Strategy: expert-parallel. The host computes routing (router matmul, softmax,
top-2, capacity dispatch — cheap integer/index work) and packs each expert's
tokens densely. Core e runs expert e's MLP (fc -> relu^2 -> proj) over its
padded token batch with fp32r matmuls. The host then gathers, weights and
combines the expert outputs.

All tensors are laid out transposed ([feature, token]) so the device kernel
needs no on-chip transposes: both matmuls contract over the partition dim.
"""
import contextlib
import ctypes
import os

import numpy as np

B, T, C, E, H, K = 4, 2048, 1024, 8, 2048, 2
N = B * T
CAP = 2 * N * K // E  # per-expert capacity; overflow tokens are dropped
NBLK = 512            # token block (matmul moving free dim)
P = 128

_NC_CACHE = {}


def _block_widths(max_kept):
    """Token-block widths covering max_kept: full 512s plus a 256 tail if it fits."""
    full, rem = divmod(max(int(max_kept), 1), NBLK)
    if rem == 0:
        return [NBLK] * full
    if rem <= NBLK // 2:
        return [NBLK] * full + [NBLK // 2]
    return [NBLK] * (full + 1)


def _build_nc(blocks, mode):
    from concourse import bacc, mybir, tile

    f32 = mybir.dt.float32
    cd = {"f32r": mybir.dt.float32r, "bf16": mybir.dt.bfloat16,
          "f16": mybir.dt.float16, "f32": mybir.dt.float32}[mode]
    io_dt = f32 if mode in ("f32r", "f32") else cd
    KC = C // P   # k-tiles for fc (contract over C)
    KH = H // P   # k-tiles for proj (contract over H)
    RT = sum(blocks)

    def rcast(ap):
        return ap.bitcast(cd) if mode == "f32r" else ap

    nc = bacc.Bacc("TRN2", target_bir_lowering=False, debug=False)
    xeT = nc.dram_tensor("xeT", [C, RT], io_dt, kind="ExternalInput")
    fcwT = nc.dram_tensor("fcwT", [C, H], io_dt, kind="ExternalInput")
    pjwT = nc.dram_tensor("pjwT", [H, C], io_dt, kind="ExternalInput")
    outT = nc.dram_tensor("outT", [C, RT], f32, kind="ExternalOutput")

    xeT_r = rcast(xeT[:].rearrange("(k p) n -> p k n", p=P))
    fcw_r = rcast(fcwT[:].rearrange("(k p) h -> p k h", p=P))
    pjw_r = rcast(pjwT[:].rearrange("(k p) c -> p k c", p=P))
    outT_r = outT[:].rearrange("(m p) n -> p m n", p=P)

    with tile.TileContext(nc) as tc:
        with (
            tc.tile_pool(name="wpool", bufs=1) as wpool,
            tc.tile_pool(name="xpool", bufs=2) as xpool,
            tc.tile_pool(name="hpool", bufs=1) as hpool,
            tc.tile_pool(name="tpool", bufs=3) as tpool,
            tc.tile_pool(name="opool", bufs=3) as opool,
            tc.tile_pool(name="pspool", bufs=8, space="PSUM") as pspool,
        ):
            fcw_sb = wpool.tile([P, KC, H], cd)
            pjw_sb = wpool.tile([P, KH, C], cd)
            x0_sb = xpool.tile([P, KC, NBLK], cd, tag="x")
            w0 = blocks[0]
            # Two parallel HWDGE streams with strict priority by queue order:
            #   scalar queue: x block 0, then proj weights (needed ~30us in)
            #   sync queue:   fc weight chunks (the first matmul's input)
            # The scalar engine stalls on DMA queue slots while issuing these,
            # so block 0 keeps its relus off the scalar engine (DVE instead).
            nc.scalar.dma_start(out=x0_sb[:, :, :w0], in_=xeT_r[:, :, 0:w0])
            for k in range(KC):
                nc.sync.dma_start(out=fcw_sb[:, k, :], in_=fcw_r[:, k, :])
            pjw_last = None
            for k in range(0, KH, 4):
                pjw_last = nc.scalar.dma_start(out=pjw_sb[:, k:k + 4, :],
                                               in_=pjw_r[:, k:k + 4, :])

            def mm1(ps, x_sb, width, m, k):
                nc.tensor.matmul(
                    ps[:, :width],
                    lhsT=fcw_sb[:, k, m * P:(m + 1) * P],
                    rhs=x_sb[:, k, :width],
                    start=(k == 0), stop=(k == KC - 1),
                )

            def mm2(ps, hid, width, m2, k2):
                nc.tensor.matmul(
                    ps[:, :width],
                    lhsT=pjw_sb[:, k2, m2 * P:(m2 + 1) * P],
                    rhs=hid[:, k2, :width],
                    start=(k2 == 0), stop=(k2 == KH - 1),
                )

            def relu_sq(ps, hid, width, m, on_dve=False):
                tmp = tpool.tile([P, NBLK], f32, tag="tmp")
                if on_dve:
                    nc.vector.tensor_scalar(
                        tmp[:, :width], ps[:, :width], 0.0, scalar2=None,
                        op0=mybir.AluOpType.max,
                    )
                else:
                    nc.scalar.activation(
                        tmp[:, :width], ps[:, :width],
                        mybir.ActivationFunctionType.Relu,
                    )
                nc.vector.tensor_tensor(
                    out=hid[:, m, :width], in0=tmp[:, :width],
                    in1=tmp[:, :width], op=mybir.AluOpType.mult,
                )

            def store(ps2, width, m2, ns):
                o_sb = opool.tile([P, NBLK], f32, tag="o")
                nc.vector.tensor_copy(o_sb[:, :width], ps2[:, :width])
                nc.sync.dma_start(out=outT_r[:, m2, ns], in_=o_sb[:, :width])

            col = 0
            for nb, width in enumerate(blocks):
                ns = slice(col, col + width)
                col += width
                if nb == 0:
                    x_sb = x0_sb
                else:
                    x_sb = xpool.tile([P, KC, NBLK], cd, tag="x")
                    x_inst = nc.gpsimd.dma_start(out=x_sb[:, :, :width],
                                                 in_=xeT_r[:, :, ns])
                    # keep prefetches from stealing DMA bandwidth during the
                    # startup weight load
                    from concourse.tile_rust import add_dep_helper
                    add_dep_helper(x_inst.ins, pjw_last.ins,
                                   reason="defer x prefetch behind weights")

                hid = hpool.tile([P, KH, NBLK], cd, tag="hid")
                if nb == 0:
                    # k-grouped: consume each weight chunk as its DMA lands,
                    # using 4 live PSUM accumulators per group.
                    for mg in range(0, KH, 4):
                        pss = [pspool.tile([P, NBLK], f32, tag="ps", name="ps")
                               for _ in range(4)]
                        for k in range(KC):
                            for j in range(4):
                                mm1(pss[j], x_sb, width, mg + j, k)
                        for j in range(4):
                            relu_sq(pss[j], hid, width, mg + j, on_dve=True)
                    for mg2 in range(0, KC, 4):
                        pss = [pspool.tile([P, NBLK], f32, tag="ps", name="ps")
                               for _ in range(4)]
                        for k2 in range(KH):
                            for j in range(4):
                                mm2(pss[j], hid, width, mg2 + j, k2)
                        for j in range(4):
                            store(pss[j], width, mg2 + j, ns)
                else:
                    for m in range(KH):
                        ps = pspool.tile([P, NBLK], f32, tag="ps")
                        for k in range(KC):
                            mm1(ps, x_sb, width, m, k)
                        relu_sq(ps, hid, width, m)
                    for m2 in range(KC):
                        ps2 = pspool.tile([P, NBLK], f32, tag="ps")
                        for k2 in range(KH):
                            mm2(ps2, hid, width, m2, k2)
                        store(ps2, width, m2, ns)

    nc.compile()
    return nc


def _profile_hook():
    """NTFF capture via libaxon ctypes (used only when MOE_PROFILE_DIR is set)."""
    so_path = "/opt/axon/libaxon_pjrt.so"
    if not os.path.exists(so_path):
        return None
    lib = ctypes.CDLL(so_path)
    if not hasattr(lib, "axon_start_nrt_profile"):
        return None
    lib.axon_start_nrt_profile.argtypes = [
        ctypes.POINTER(ctypes.c_int64), ctypes.c_size_t,
    ]
    lib.axon_start_nrt_profile.restype = ctypes.c_int64
    lib.axon_stop_nrt_profile.argtypes = [ctypes.c_char_p]
    lib.axon_stop_nrt_profile.restype = ctypes.c_int64

    @contextlib.contextmanager
    def _hook(output_dir, device_ids):
        import jax
        jax.devices()
        ids = (ctypes.c_int64 * len(device_ids))(*device_ids)
        rc = lib.axon_start_nrt_profile(ids, len(device_ids))
        if rc != 0:
            raise RuntimeError(f"axon_start_nrt_profile rc={rc}")
        try:
            yield
        finally:
            n = lib.axon_stop_nrt_profile(str(output_dir).encode())
            print(f"profile: {n} file(s) written to {output_dir}")

    return _hook


def _run_device(xeT, fcwT_all, pjwT_all, blocks, mode):
    from concourse.bass_utils import run_bass_kernel_spmd

    key = (tuple(blocks), mode)
    if key not in _NC_CACHE:
        _NC_CACHE[key] = _build_nc(blocks, mode)
    nc = _NC_CACHE[key]

    in_maps = [
        {"xeT": xeT[e], "fcwT": fcwT_all[e], "pjwT": pjwT_all[e]}
        for e in range(E)
    ]
    core_ids = list(range(E))

    prof_dir = os.environ.get("MOE_PROFILE_DIR")
    if prof_dir:
        hook = _profile_hook()
        # warm-up run first so the profiled run measures a warm device
        run_bass_kernel_spmd(nc, in_maps, core_ids)
        with hook(prof_dir, core_ids):
            res = run_bass_kernel_spmd(nc, in_maps, core_ids)
    else:
        res = run_bass_kernel_spmd(nc, in_maps, core_ids)

    return np.stack([res.results[e]["outT"] for e in range(E)])


def kernel(x, router_w, fc_w, proj_w):
    x = np.asarray(x, np.float32)
    router_w = np.asarray(router_w, np.float32)
    fc_w = np.asarray(fc_w, np.float32)
    proj_w = np.asarray(proj_w, np.float32)

    mode = os.environ.get("MOE_DTYPE", "f32r")
    x_flat = x.reshape(N, C)

    # --- routing (mirrors the reference numerics in f32) ---
    scores = x_flat @ router_w.T                        # [N, E]
    mx = scores.max(-1, keepdims=True)
    ex = np.exp(scores - mx)
    probs = (ex / ex.sum(-1, keepdims=True)).astype(np.float32)
    top_idx = np.argsort(-probs, axis=-1, kind="stable")[:, :K]
    top_w = np.take_along_axis(probs, top_idx, -1)
    top_w = top_w / (top_w.sum(-1, keepdims=True) + 1e-10)

    fe = top_idx.reshape(-1)                            # [N*K]
    fw = top_w.reshape(-1).astype(np.float32)
    ft = np.repeat(np.arange(N), K)
    order = np.argsort(fe, kind="stable")
    se, st = fe[order], ft[order]
    counts = np.bincount(fe, minlength=E)
    offs = np.concatenate([[0], np.cumsum(counts)[:-1]]).astype(np.int64)
    pos_sorted = np.arange(N * K) - offs[se]
    kept = np.minimum(counts, CAP)

    blocks = _block_widths(kept.max())
    RT = sum(blocks)

    if mode == "bf16":
        import ml_dtypes
        io_np = ml_dtypes.bfloat16
    elif mode == "f16":
        io_np = np.float16
    else:
        io_np = np.float32

    # --- pack per-expert token batches, transposed ---
    xeT = np.zeros((E, C, RT), io_np)
    for e in range(E):
        toks = st[offs[e]: offs[e] + kept[e]]
        xeT[e, :, :kept[e]] = x_flat[toks].T.astype(io_np)
    fcwT_all = np.ascontiguousarray(
        fc_w.transpose(0, 2, 1)).astype(io_np)    # [E, C, H]
    pjwT_all = np.ascontiguousarray(
        proj_w.transpose(0, 2, 1)).astype(io_np)  # [E, H, C]

    outT = _run_device(xeT, fcwT_all, pjwT_all, blocks, mode)  # [E, C, RT]

    # --- combine: weight each (token, slot) contribution and sum ---
    inv = np.empty(N * K, np.int64)
    inv[order] = np.arange(N * K)
    pos_flat = pos_sorted[inv]
    valid = pos_flat < CAP
    pos_c = np.where(valid, pos_flat, 0)
    w_eff = np.where(valid, fw, 0.0).astype(np.float32)

    gathered = outT[fe, :, pos_c]                              # [N*K, C]
    out = (gathered * w_eff[:, None]).reshape(N, K, C).sum(1)

    return (
        out.reshape(B, T, C).astype(np.float32),
        probs.reshape(B, T, E).astype(np.float32),
    )


# revision 17
# speedup vs baseline: 1.2330x; 1.0604x over previous
"""MoE layer (top-2 routing, 8 experts) on 8 TRN2 NeuronCores.

Strategy: expert-parallel. The host computes routing (router matmul, softmax,
top-2, capacity dispatch — cheap integer/index work) and packs each expert's
tokens densely. Core e runs expert e's MLP (fc -> relu^2 -> proj) over its
padded token batch with fp32r matmuls. The host then gathers, weights and
combines the expert outputs.

All tensors are laid out transposed ([feature, token]) so the device kernel
needs no on-chip transposes: both matmuls contract over the partition dim.
"""
import contextlib
import ctypes
import os

import numpy as np

B, T, C, E, H, K = 4, 2048, 1024, 8, 2048, 2
N = B * T
CAP = 2 * N * K // E  # per-expert capacity; overflow tokens are dropped
NBLK = 512            # token block (matmul moving free dim)
P = 128

_NC_CACHE = {}


def _block_widths(max_kept):
    """Token-block widths covering max_kept: full 512s plus a narrower tail."""
    full, rem = divmod(max(int(max_kept), 1), NBLK)
    widths = [NBLK] * full
    if rem:
        widths.append(((rem + P - 1) // P) * P)
    return widths


def _build_nc(blocks, mode):
    from concourse import bacc, mybir, tile

    f32 = mybir.dt.float32
    cd = {"f32r": mybir.dt.float32r, "bf16": mybir.dt.bfloat16,
          "f16": mybir.dt.float16, "f32": mybir.dt.float32}[mode]
    io_dt = f32 if mode in ("f32r", "f32") else cd
    KC = C // P   # k-tiles for fc (contract over C)
    KH = H // P   # k-tiles for proj (contract over H)
    RT = sum(blocks)

    def rcast(ap):
        return ap.bitcast(cd) if mode == "f32r" else ap

    nc = bacc.Bacc("TRN2", target_bir_lowering=False, debug=False)
    xeT = nc.dram_tensor("xeT", [C, RT], io_dt, kind="ExternalInput")
    fcwT = nc.dram_tensor("fcwT", [C, H], io_dt, kind="ExternalInput")
    pjwT = nc.dram_tensor("pjwT", [H, C], io_dt, kind="ExternalInput")
    outT = nc.dram_tensor("outT", [C, RT], f32, kind="ExternalOutput")

    xeT_r = rcast(xeT[:].rearrange("(k p) n -> p k n", p=P))
    fcw_r = rcast(fcwT[:].rearrange("(k p) h -> p k h", p=P))
    pjw_r = rcast(pjwT[:].rearrange("(k p) c -> p k c", p=P))
    outT_r = outT[:].rearrange("(m p) n -> p m n", p=P)

    with tile.TileContext(nc) as tc:
        with (
            tc.tile_pool(name="wpool", bufs=1) as wpool,
            tc.tile_pool(name="xpool", bufs=2) as xpool,
            tc.tile_pool(name="hpool", bufs=1) as hpool,
            tc.tile_pool(name="tpool", bufs=3) as tpool,
            tc.tile_pool(name="opool", bufs=3) as opool,
            tc.tile_pool(name="pspool", bufs=8, space="PSUM") as pspool,
        ):
            fcw_sb = wpool.tile([P, KC, H], cd)
            pjw_sb = wpool.tile([P, KH, C], cd)
            x0_sb = xpool.tile([P, KC, NBLK], cd, tag="x")
            w0 = blocks[0]
            # Two parallel HWDGE streams with strict priority by queue order:
            #   scalar queue: x block 0, then proj weights (needed ~30us in)
            #   sync queue:   fc weight chunks (the first matmul's input)
            # The scalar engine stalls on DMA queue slots while issuing these,
            # so block 0 keeps its relus off the scalar engine (DVE instead).
            nc.scalar.dma_start(out=x0_sb[:, 0, :w0], in_=xeT_r[:, 0, 0:w0])
            nc.scalar.dma_start(out=x0_sb[:, 1:, :w0], in_=xeT_r[:, 1:, 0:w0])
            for k in range(KC):
                nc.sync.dma_start(out=fcw_sb[:, k, :], in_=fcw_r[:, k, :])
            # dummy matmuls on freshly-memset tiles: keeps the PE busy during
            # the weight DMA so the HAM clock gate is warm (2.4 GHz) when the
            # real stream starts
            warm_w = tpool.tile([P, P], cd, tag="warm_w", bufs=1)
            warm_x = tpool.tile([P, NBLK], cd, tag="warm_x", bufs=1)
            nc.vector.memset(warm_w[:], 0.0)
            nc.vector.memset(warm_x[:], 0.0)
            warm_ps = pspool.tile([P, NBLK], f32, tag="ps")
            for _ in range(14):
                nc.tensor.matmul(warm_ps[:], lhsT=warm_w[:], rhs=warm_x[:],
                                 start=True, stop=True)
            pjw_last = None
            for k in range(0, KH, 4):
                pjw_last = nc.scalar.dma_start(out=pjw_sb[:, k:k + 4, :],
                                               in_=pjw_r[:, k:k + 4, :])

            def mm1(ps, x_sb, width, m, k):
                nc.tensor.matmul(
                    ps[:, :width],
                    lhsT=fcw_sb[:, k, m * P:(m + 1) * P],
                    rhs=x_sb[:, k, :width],
                    start=(k == 0), stop=(k == KC - 1),
                )

            def mm2(ps, hid, width, m2, k2):
                nc.tensor.matmul(
                    ps[:, :width],
                    lhsT=pjw_sb[:, k2, m2 * P:(m2 + 1) * P],
                    rhs=hid[:, k2, :width],
                    start=(k2 == 0), stop=(k2 == KH - 1),
                )

            def relu_sq(ps, hid, width, m, on_dve=False):
                tmp = tpool.tile([P, NBLK], f32, tag="tmp")
                if on_dve:
                    nc.vector.tensor_scalar(
                        tmp[:, :width], ps[:, :width], 0.0, scalar2=None,
                        op0=mybir.AluOpType.max,
                    )
                else:
                    nc.scalar.activation(
                        tmp[:, :width], ps[:, :width],
                        mybir.ActivationFunctionType.Relu,
                    )
                nc.vector.tensor_tensor(
                    out=hid[:, m, :width], in0=tmp[:, :width],
                    in1=tmp[:, :width], op=mybir.AluOpType.mult,
                )

            def store(ps2, width, m2, ns):
                o_sb = opool.tile([P, NBLK], f32, tag="o")
                nc.vector.tensor_copy(o_sb[:, :width], ps2[:, :width])
                nc.sync.dma_start(out=outT_r[:, m2, ns], in_=o_sb[:, :width])

            col = 0
            for nb, width in enumerate(blocks):
                ns = slice(col, col + width)
                col += width
                if nb == 0:
                    x_sb = x0_sb
                else:
                    x_sb = xpool.tile([P, KC, NBLK], cd, tag="x")
                    x_inst = nc.gpsimd.dma_start(out=x_sb[:, :, :width],
                                                 in_=xeT_r[:, :, ns])
                    # keep prefetches from stealing DMA bandwidth during the
                    # startup weight load
                    from concourse.tile_rust import add_dep_helper
                    add_dep_helper(x_inst.ins, pjw_last.ins,
                                   reason="defer x prefetch behind weights")

                hid = hpool.tile([P, KH, NBLK], cd, tag="hid")
                if nb == 0:
                    # k-grouped: consume each weight chunk as its DMA lands,
                    # using 4 live PSUM accumulators per group.
                    for mg in range(0, KH, 4):
                        pss = [pspool.tile([P, NBLK], f32, tag="ps", name="ps")
                               for _ in range(4)]
                        for k in range(KC):
                            for j in range(4):
                                mm1(pss[j], x_sb, width, mg + j, k)
                        for j in range(4):
                            relu_sq(pss[j], hid, width, mg + j, on_dve=True)
                    for mg2 in range(0, KC, 4):
                        pss = [pspool.tile([P, NBLK], f32, tag="ps", name="ps")
                               for _ in range(4)]
                        for k2 in range(KH):
                            for j in range(4):
                                mm2(pss[j], hid, width, mg2 + j, k2)
                        for j in range(4):
                            store(pss[j], width, mg2 + j, ns)
                else:
                    for m in range(KH):
                        ps = pspool.tile([P, NBLK], f32, tag="ps")
                        for k in range(KC):
                            mm1(ps, x_sb, width, m, k)
                        relu_sq(ps, hid, width, m)
                    for m2 in range(KC):
                        ps2 = pspool.tile([P, NBLK], f32, tag="ps")
                        for k2 in range(KH):
                            mm2(ps2, hid, width, m2, k2)
                        store(ps2, width, m2, ns)

    nc.compile()
    return nc


def _profile_hook():
    """NTFF capture via libaxon ctypes (used only when MOE_PROFILE_DIR is set)."""
    so_path = "/opt/axon/libaxon_pjrt.so"
    if not os.path.exists(so_path):
        return None
    lib = ctypes.CDLL(so_path)
    if not hasattr(lib, "axon_start_nrt_profile"):
        return None
    lib.axon_start_nrt_profile.argtypes = [
        ctypes.POINTER(ctypes.c_int64), ctypes.c_size_t,
    ]
    lib.axon_start_nrt_profile.restype = ctypes.c_int64
    lib.axon_stop_nrt_profile.argtypes = [ctypes.c_char_p]
    lib.axon_stop_nrt_profile.restype = ctypes.c_int64

    @contextlib.contextmanager
    def _hook(output_dir, device_ids):
        import jax
        jax.devices()
        ids = (ctypes.c_int64 * len(device_ids))(*device_ids)
        rc = lib.axon_start_nrt_profile(ids, len(device_ids))
        if rc != 0:
            raise RuntimeError(f"axon_start_nrt_profile rc={rc}")
        try:
            yield
        finally:
            n = lib.axon_stop_nrt_profile(str(output_dir).encode())
            print(f"profile: {n} file(s) written to {output_dir}")

    return _hook


def _run_device(xeT, fcwT_all, pjwT_all, blocks, mode):
    from concourse.bass_utils import run_bass_kernel_spmd

    key = (tuple(blocks), mode)
    if key not in _NC_CACHE:
        _NC_CACHE[key] = _build_nc(blocks, mode)
    nc = _NC_CACHE[key]

    in_maps = [
        {"xeT": xeT[e], "fcwT": fcwT_all[e], "pjwT": pjwT_all[e]}
        for e in range(E)
    ]
    core_ids = list(range(E))

    prof_dir = os.environ.get("MOE_PROFILE_DIR")
    if prof_dir:
        hook = _profile_hook()
        # warm-up run first so the profiled run measures a warm device
        run_bass_kernel_spmd(nc, in_maps, core_ids)
        with hook(prof_dir, core_ids):
            res = run_bass_kernel_spmd(nc, in_maps, core_ids)
    else:
        res = run_bass_kernel_spmd(nc, in_maps, core_ids)

    return np.stack([res.results[e]["outT"] for e in range(E)])


def kernel(x, router_w, fc_w, proj_w):
    x = np.asarray(x, np.float32)
    router_w = np.asarray(router_w, np.float32)
    fc_w = np.asarray(fc_w, np.float32)
    proj_w = np.asarray(proj_w, np.float32)

    mode = os.environ.get("MOE_DTYPE", "f32r")
    x_flat = x.reshape(N, C)

    # --- routing (mirrors the reference numerics in f32) ---
    scores = x_flat @ router_w.T                        # [N, E]
    mx = scores.max(-1, keepdims=True)
    ex = np.exp(scores - mx)
    probs = (ex / ex.sum(-1, keepdims=True)).astype(np.float32)
    top_idx = np.argsort(-probs, axis=-1, kind="stable")[:, :K]
    top_w = np.take_along_axis(probs, top_idx, -1)
    top_w = top_w / (top_w.sum(-1, keepdims=True) + 1e-10)

    fe = top_idx.reshape(-1)                            # [N*K]
    fw = top_w.reshape(-1).astype(np.float32)
    ft = np.repeat(np.arange(N), K)
    order = np.argsort(fe, kind="stable")
    se, st = fe[order], ft[order]
    counts = np.bincount(fe, minlength=E)
    offs = np.concatenate([[0], np.cumsum(counts)[:-1]]).astype(np.int64)
    pos_sorted = np.arange(N * K) - offs[se]
    kept = np.minimum(counts, CAP)

    blocks = _block_widths(kept.max())
    RT = sum(blocks)

    if mode == "bf16":
        import ml_dtypes
        io_np = ml_dtypes.bfloat16
    elif mode == "f16":
        io_np = np.float16
    else:
        io_np = np.float32

    # --- pack per-expert token batches, transposed ---
    xeT = np.zeros((E, C, RT), io_np)
    for e in range(E):
        toks = st[offs[e]: offs[e] + kept[e]]
        xeT[e, :, :kept[e]] = x_flat[toks].T.astype(io_np)
    fcwT_all = np.ascontiguousarray(
        fc_w.transpose(0, 2, 1)).astype(io_np)    # [E, C, H]
    pjwT_all = np.ascontiguousarray(
        proj_w.transpose(0, 2, 1)).astype(io_np)  # [E, H, C]

    outT = _run_device(xeT, fcwT_all, pjwT_all, blocks, mode)  # [E, C, RT]

    # --- combine: weight each (token, slot) contribution and sum ---
    inv = np.empty(N * K, np.int64)
    inv[order] = np.arange(N * K)
    pos_flat = pos_sorted[inv]
    valid = pos_flat < CAP
    pos_c = np.where(valid, pos_flat, 0)
    w_eff = np.where(valid, fw, 0.0).astype(np.float32)

    gathered = outT[fe, :, pos_c]                              # [N*K, C]
    out = (gathered * w_eff[:, None]).reshape(N, K, C).sum(1)

    return (
        out.reshape(B, T, C).astype(np.float32),
        probs.reshape(B, T, E).astype(np.float32),
    )


# revision 22
# speedup vs baseline: 1.2363x; 1.0027x over previous
"""MoE layer (top-2 routing, 8 experts) on 8 TRN2 NeuronCores.

Strategy: expert-parallel. The host computes routing (router matmul, softmax,
top-2, capacity dispatch — cheap integer/index work) and packs each expert's
tokens densely. Core e runs expert e's MLP (fc -> relu^2 -> proj) over its
padded token batch with fp32r matmuls. The host then gathers, weights and
combines the expert outputs.

All tensors are laid out transposed ([feature, token]) so the device kernel
needs no on-chip transposes: both matmuls contract over the partition dim.
"""
import contextlib
import ctypes
import os

import numpy as np

B, T, C, E, H, K = 4, 2048, 1024, 8, 2048, 2
N = B * T
CAP = 2 * N * K // E  # per-expert capacity; overflow tokens are dropped
NBLK = 512            # token block (matmul moving free dim)
P = 128

_NC_CACHE = {}


def _block_widths(max_kept):
    """Token-block widths covering max_kept: full 512s plus a narrower tail."""
    full, rem = divmod(max(int(max_kept), 1), NBLK)
    widths = [NBLK] * full
    if rem:
        widths.append(((rem + P - 1) // P) * P)
    return widths


def _build_nc(blocks, mode):
    from concourse import bacc, mybir, tile

    f32 = mybir.dt.float32
    cd = {"f32r": mybir.dt.float32r, "bf16": mybir.dt.bfloat16,
          "f16": mybir.dt.float16, "f32": mybir.dt.float32}[mode]
    io_dt = f32 if mode in ("f32r", "f32") else cd
    KC = C // P   # k-tiles for fc (contract over C)
    KH = H // P   # k-tiles for proj (contract over H)
    RT = sum(blocks)

    def rcast(ap):
        return ap.bitcast(cd) if mode == "f32r" else ap

    nc = bacc.Bacc("TRN2", target_bir_lowering=False, debug=False)
    xeT = nc.dram_tensor("xeT", [C, RT], io_dt, kind="ExternalInput")
    fcwT = nc.dram_tensor("fcwT", [C, H], io_dt, kind="ExternalInput")
    pjwT = nc.dram_tensor("pjwT", [H, C], io_dt, kind="ExternalInput")
    outT = nc.dram_tensor("outT", [C, RT], f32, kind="ExternalOutput")

    xeT_r = rcast(xeT[:].rearrange("(k p) n -> p k n", p=P))
    fcw_r = rcast(fcwT[:].rearrange("(k p) h -> p k h", p=P))
    pjw_r = rcast(pjwT[:].rearrange("(k p) c -> p k c", p=P))
    outT_r = outT[:].rearrange("(m p) n -> p m n", p=P)

    # pair the narrow tail block with its predecessor (one weight load feeds
    # both matmuls); needs two live hid tiles, which only fits with 2-byte types
    pair_tail = (len(blocks) >= 3 and blocks[-1] < NBLK
                 and mode in ("f16", "bf16"))

    with tile.TileContext(nc) as tc:
        with (
            tc.tile_pool(name="wpool", bufs=1) as wpool,
            tc.tile_pool(name="xpool", bufs=2) as xpool,
            tc.tile_pool(name="hpool", bufs=2 if pair_tail else 1) as hpool,
            tc.tile_pool(name="tpool", bufs=3) as tpool,
            tc.tile_pool(name="opool", bufs=3) as opool,
            tc.tile_pool(name="pspool", bufs=8, space="PSUM") as pspool,
        ):
            fcw_sb = wpool.tile([P, KC, H], cd)
            pjw_sb = wpool.tile([P, KH, C], cd)
            x0_sb = xpool.tile([P, KC, NBLK], cd, tag="x")
            w0 = blocks[0]
            # Two parallel HWDGE streams with strict priority by queue order:
            #   scalar queue: x block 0, then proj weights (needed ~30us in)
            #   sync queue:   fc weight chunks (the first matmul's input)
            # The scalar engine stalls on DMA queue slots while issuing these,
            # so block 0 keeps its relus off the scalar engine (DVE instead).
            nc.scalar.dma_start(out=x0_sb[:, 0, :w0], in_=xeT_r[:, 0, 0:w0])
            nc.scalar.dma_start(out=x0_sb[:, 1:, :w0], in_=xeT_r[:, 1:, 0:w0])
            for k in range(KC):
                nc.sync.dma_start(out=fcw_sb[:, k, :], in_=fcw_r[:, k, :])
            # dummy matmuls on freshly-memset tiles: keeps the PE busy during
            # the weight DMA so the HAM clock gate is warm (2.4 GHz) when the
            # real stream starts
            warm_w = tpool.tile([P, P], cd, tag="warm_w", bufs=1)
            warm_x = tpool.tile([P, NBLK], cd, tag="warm_x", bufs=1)
            nc.vector.memset(warm_w[:], 0.0)
            nc.vector.memset(warm_x[:], 0.0)
            warm_ps = pspool.tile([P, NBLK], f32, tag="ps")
            for _ in range(14):
                nc.tensor.matmul(warm_ps[:], lhsT=warm_w[:], rhs=warm_x[:],
                                 start=True, stop=True)
            pjw_last = None
            for k in range(0, KH, 4):
                pjw_last = nc.scalar.dma_start(out=pjw_sb[:, k:k + 4, :],
                                               in_=pjw_r[:, k:k + 4, :])

            def mm1(ps, x_sb, width, m, k):
                nc.tensor.matmul(
                    ps[:, :width],
                    lhsT=fcw_sb[:, k, m * P:(m + 1) * P],
                    rhs=x_sb[:, k, :width],
                    start=(k == 0), stop=(k == KC - 1),
                )

            def mm2(ps, hid, width, m2, k2):
                nc.tensor.matmul(
                    ps[:, :width],
                    lhsT=pjw_sb[:, k2, m2 * P:(m2 + 1) * P],
                    rhs=hid[:, k2, :width],
                    start=(k2 == 0), stop=(k2 == KH - 1),
                )

            def relu_sq(ps, hid, width, m, on_dve=False):
                tmp = tpool.tile([P, NBLK], f32, tag="tmp")
                if on_dve:
                    nc.vector.tensor_scalar(
                        tmp[:, :width], ps[:, :width], 0.0, scalar2=None,
                        op0=mybir.AluOpType.max,
                    )
                else:
                    nc.scalar.activation(
                        tmp[:, :width], ps[:, :width],
                        mybir.ActivationFunctionType.Relu,
                    )
                nc.vector.tensor_tensor(
                    out=hid[:, m, :width], in0=tmp[:, :width],
                    in1=tmp[:, :width], op=mybir.AluOpType.mult,
                )

            def store(ps2, width, m2, ns):
                o_sb = opool.tile([P, NBLK], f32, tag="o")
                nc.vector.tensor_copy(o_sb[:, :width], ps2[:, :width])
                nc.sync.dma_start(out=outT_r[:, m2, ns], in_=o_sb[:, :width])

            def load_x(width, ns):
                x_sb = xpool.tile([P, KC, NBLK], cd, tag="x", name="x_sb")
                x_inst = nc.gpsimd.dma_start(out=x_sb[:, :, :width],
                                             in_=xeT_r[:, :, ns])
                # keep prefetches from stealing DMA bandwidth during the
                # startup weight load
                from concourse.tile_rust import add_dep_helper
                add_dep_helper(x_inst.ins, pjw_last.ins,
                               reason="defer x prefetch behind weights")
                return x_sb

            col = 0
            nb = 0
            while nb < len(blocks):
                width = blocks[nb]
                ns = slice(col, col + width)
                col += width
                if pair_tail and nb == len(blocks) - 2:
                    # paired tail: one weight tile feeds a matmul for each of
                    # the last two blocks, amortizing LDWEIGHTS over the
                    # narrow tail block
                    wb = blocks[nb + 1]
                    nsb = slice(col, col + wb)
                    col += wb
                    x_a = load_x(width, ns)
                    x_b = load_x(wb, nsb)
                    hid_a = hpool.tile([P, KH, NBLK], cd, tag="hid",
                                       name="hid_a")
                    hid_b = hpool.tile([P, KH, NBLK], cd, tag="hid",
                                       name="hid_b")
                    for m in range(KH):
                        ps_a = pspool.tile([P, NBLK], f32, tag="ps",
                                           name="ps_a")
                        ps_b = pspool.tile([P, NBLK], f32, tag="ps",
                                           name="ps_b")
                        for k in range(KC):
                            mm1(ps_a, x_a, width, m, k)
                            mm1(ps_b, x_b, wb, m, k)
                        relu_sq(ps_a, hid_a, width, m)
                        relu_sq(ps_b, hid_b, wb, m)
                    for m2 in range(KC):
                        ps2_a = pspool.tile([P, NBLK], f32, tag="ps",
                                            name="ps2_a")
                        ps2_b = pspool.tile([P, NBLK], f32, tag="ps",
                                            name="ps2_b")
                        for k2 in range(KH):
                            mm2(ps2_a, hid_a, width, m2, k2)
                            mm2(ps2_b, hid_b, wb, m2, k2)
                        store(ps2_a, width, m2, ns)
                        store(ps2_b, wb, m2, nsb)
                    nb += 2
                    continue

                if nb == 0:
                    x_sb = x0_sb
                else:
                    x_sb = load_x(width, ns)

                hid = hpool.tile([P, KH, NBLK], cd, tag="hid")
                if nb == 0:
                    # k-grouped: consume each weight chunk as its DMA lands,
                    # using 4 live PSUM accumulators per group.
                    for mg in range(0, KH, 4):
                        pss = [pspool.tile([P, NBLK], f32, tag="ps", name="ps")
                               for _ in range(4)]
                        for k in range(KC):
                            for j in range(4):
                                mm1(pss[j], x_sb, width, mg + j, k)
                        for j in range(4):
                            relu_sq(pss[j], hid, width, mg + j, on_dve=True)
                    for mg2 in range(0, KC, 4):
                        pss = [pspool.tile([P, NBLK], f32, tag="ps", name="ps")
                               for _ in range(4)]
                        for k2 in range(KH):
                            for j in range(4):
                                mm2(pss[j], hid, width, mg2 + j, k2)
                        for j in range(4):
                            store(pss[j], width, mg2 + j, ns)
                else:
                    for m in range(KH):
                        ps = pspool.tile([P, NBLK], f32, tag="ps")
                        for k in range(KC):
                            mm1(ps, x_sb, width, m, k)
                        relu_sq(ps, hid, width, m)
                    for m2 in range(KC):
                        ps2 = pspool.tile([P, NBLK], f32, tag="ps")
                        for k2 in range(KH):
                            mm2(ps2, hid, width, m2, k2)
                        store(ps2, width, m2, ns)
                nb += 1

    nc.compile()
    return nc


def _profile_hook():
    """NTFF capture via libaxon ctypes (used only when MOE_PROFILE_DIR is set)."""
    so_path = "/opt/axon/libaxon_pjrt.so"
    if not os.path.exists(so_path):
        return None
    lib = ctypes.CDLL(so_path)
    if not hasattr(lib, "axon_start_nrt_profile"):
        return None
    lib.axon_start_nrt_profile.argtypes = [
        ctypes.POINTER(ctypes.c_int64), ctypes.c_size_t,
    ]
    lib.axon_start_nrt_profile.restype = ctypes.c_int64
    lib.axon_stop_nrt_profile.argtypes = [ctypes.c_char_p]
    lib.axon_stop_nrt_profile.restype = ctypes.c_int64

    @contextlib.contextmanager
    def _hook(output_dir, device_ids):
        import jax
        jax.devices()
        ids = (ctypes.c_int64 * len(device_ids))(*device_ids)
        rc = lib.axon_start_nrt_profile(ids, len(device_ids))
        if rc != 0:
            raise RuntimeError(f"axon_start_nrt_profile rc={rc}")
        try:
            yield
        finally:
            n = lib.axon_stop_nrt_profile(str(output_dir).encode())
            print(f"profile: {n} file(s) written to {output_dir}")

    return _hook


def _ensure_axon_hooks():
    """bass_utils' BASS_TRACE path imports antenv.axon_hooks, which some
    images lack. Install a small shim wired to the libaxon ctypes hook so
    trace=True degrades gracefully (or works, when the .so supports it)."""
    try:
        import antenv.axon_hooks  # noqa: F401
        return
    except ImportError:
        pass
    import sys
    import types
    try:
        import antenv
    except ImportError:
        return
    mod = types.ModuleType("antenv.axon_hooks")
    mod._hook = None

    def set_axon_ntff_profile_hook(h):
        mod._hook = h

    def get_axon_ntff_profile_hook():
        if mod._hook is None:
            try:
                mod._hook = _profile_hook()
            except Exception:
                mod._hook = None
        return mod._hook

    mod.set_axon_ntff_profile_hook = set_axon_ntff_profile_hook
    mod.get_axon_ntff_profile_hook = get_axon_ntff_profile_hook
    sys.modules["antenv.axon_hooks"] = mod
    antenv.axon_hooks = mod


def _run_device(xeT, fcwT_all, pjwT_all, blocks, mode):
    _ensure_axon_hooks()
    from concourse.bass_utils import run_bass_kernel_spmd

    key = (tuple(blocks), mode)
    if key not in _NC_CACHE:
        _NC_CACHE[key] = _build_nc(blocks, mode)
    nc = _NC_CACHE[key]

    in_maps = [
        {"xeT": xeT[e], "fcwT": fcwT_all[e], "pjwT": pjwT_all[e]}
        for e in range(E)
    ]
    core_ids = list(range(E))

    prof_dir = os.environ.get("MOE_PROFILE_DIR")
    if prof_dir:
        hook = _profile_hook()
        # warm-up run first so the profiled run measures a warm device
        run_bass_kernel_spmd(nc, in_maps, core_ids)
        with hook(prof_dir, core_ids):
            res = run_bass_kernel_spmd(nc, in_maps, core_ids)
    else:
        res = run_bass_kernel_spmd(nc, in_maps, core_ids)

    return np.stack([res.results[e]["outT"] for e in range(E)])


def kernel(x, router_w, fc_w, proj_w):
    x = np.asarray(x, np.float32)
    router_w = np.asarray(router_w, np.float32)
    fc_w = np.asarray(fc_w, np.float32)
    proj_w = np.asarray(proj_w, np.float32)

    mode = os.environ.get("MOE_DTYPE", "f32r")
    x_flat = x.reshape(N, C)

    # --- routing (mirrors the reference numerics in f32) ---
    scores = x_flat @ router_w.T                        # [N, E]
    mx = scores.max(-1, keepdims=True)
    ex = np.exp(scores - mx)
    probs = (ex / ex.sum(-1, keepdims=True)).astype(np.float32)
    top_idx = np.argsort(-probs, axis=-1, kind="stable")[:, :K]
    top_w = np.take_along_axis(probs, top_idx, -1)
    top_w = top_w / (top_w.sum(-1, keepdims=True) + 1e-10)

    fe = top_idx.reshape(-1)                            # [N*K]
    fw = top_w.reshape(-1).astype(np.float32)
    ft = np.repeat(np.arange(N), K)
    order = np.argsort(fe, kind="stable")
    se, st = fe[order], ft[order]
    counts = np.bincount(fe, minlength=E)
    offs = np.concatenate([[0], np.cumsum(counts)[:-1]]).astype(np.int64)
    pos_sorted = np.arange(N * K) - offs[se]
    kept = np.minimum(counts, CAP)

    blocks = _block_widths(kept.max())
    RT = sum(blocks)

    if mode == "bf16":
        import ml_dtypes
        io_np = ml_dtypes.bfloat16
    elif mode == "f16":
        io_np = np.float16
    else:
        io_np = np.float32

    # --- pack per-expert token batches, transposed ---
    xeT = np.zeros((E, C, RT), io_np)
    for e in range(E):
        toks = st[offs[e]: offs[e] + kept[e]]
        xeT[e, :, :kept[e]] = x_flat[toks].T.astype(io_np)
    fcwT_all = np.ascontiguousarray(
        fc_w.transpose(0, 2, 1)).astype(io_np)    # [E, C, H]
    pjwT_all = np.ascontiguousarray(
        proj_w.transpose(0, 2, 1)).astype(io_np)  # [E, H, C]

    outT = _run_device(xeT, fcwT_all, pjwT_all, blocks, mode)  # [E, C, RT]

    # --- combine: weight each (token, slot) contribution and sum ---
    inv = np.empty(N * K, np.int64)
    inv[order] = np.arange(N * K)
    pos_flat = pos_sorted[inv]
    valid = pos_flat < CAP
    pos_c = np.where(valid, pos_flat, 0)
    w_eff = np.where(valid, fw, 0.0).astype(np.float32)

    gathered = outT[fe, :, pos_c]                              # [N*K, C]
    out = (gathered * w_eff[:, None]).reshape(N, K, C).sum(1)

    return (
        out.reshape(B, T, C).astype(np.float32),
        probs.reshape(B, T, E).astype(np.float32),
    )
